# revision 3
# baseline (speedup 1.0000x reference)
"""Trainium2 Bass kernel for the KerasArima 2nd-order linear recurrence.

Reference computes, per lane (b, h, w):
    y_t = x_t + phi*(x_t - x_{t-1}) - theta_1*(x_t - y_{t-1}) - theta_2*(x_{t-1} - y_{t-2})
which is a linear constant-coefficient recurrence
    y_t = a*x_t + b*x_{t-1} + c*y_{t-1} + d*y_{t-2}
with a = 1+phi-theta_1, b = -(phi+theta_2), c = theta_1, d = theta_2.
Because |c|,|d| ~ 0.1, the impulse response decays below fp32 eps within ~40
lags, so y is exactly (to fp32) a short causal convolution of x. Blocked into
128-step time blocks this becomes two dense 128x128 Toeplitz matmuls per block:
    Y_blk = M0 @ X_blk + M1 @ X_{blk-1}
with first-block initial conditions folded into a modified M0 (column 0) plus a
per-timestep bias vector. All matmuls run on the TensorEngine in fp32; lanes
(batch x H x W) ride the matmul free dimension.

Sharding: pure data parallelism - batch axis split 8 ways across NeuronCores.
"""

import numpy as np

# Problem shape (hardcoded per contract)
B, T, H, W = 64, 2048, 16, 16
LANES = H * W                # 256
NCORES = 8
BPC = B // NCORES            # 8 batches per core
P = 128                      # time-block size = partition count
NBLK = T // P                # 16 blocks per batch
PAIR = 2                     # batches fused into one matmul free dim
SUPER = 4                    # time blocks per DMA superblock
FREE = PAIR * LANES          # 512 = fp32 moving-operand max
NPAIR = BPC // PAIR          # 4
NSUP = NBLK // SUPER         # 4

_cache = {}


def _coeffs(phi, t1, t2, e0):
    """Host-side (float64) Toeplitz block matrices + first-block bias."""
    a = 1.0 + phi - t1
    b = -(phi + t2)
    c, d = t1, t2
    K = 2 * P
    h = np.zeros(K + 1)
    h[0] = 1.0
    h[1] = c
    for k in range(2, K + 1):
        h[k] = c * h[k - 1] + d * h[k - 2]
    g = np.zeros(K + 1)
    g[0] = a
    g[1:] = a * h[1:] + b * h[:-1]

    M0 = np.zeros((P, P))
    for j in range(P):
        M0[j:, j] = g[:P - j]
    M1 = np.zeros((P, P))
    for j in range(P):
        M1[:, j] = g[P - j:2 * P - j]

    # Initial-condition corrections (zero-state -> true y_0, y_1):
    #   delta0 = (t1-phi)*x_0 - t1*e0 ; delta1 = t2*(x_0 - e0)
    # y_t += h_t*delta0 + h_{t-1}*delta1  for t in [0, P)
    q = np.zeros(P)
    r = np.zeros(P)
    q[0] = t1 - phi
    r[0] = -e0 * t1
    q[1:] = (t1 - phi) * h[1:P] + t2 * h[:P - 1]
    r[1:] = -e0 * (t1 * h[1:P] + t2 * h[:P - 1])
    M0f = M0.copy()
    M0f[:, 0] += q
    return (
        np.ascontiguousarray(M0.T, np.float32),
        np.ascontiguousarray(M1.T, np.float32),
        np.ascontiguousarray(M0f.T, np.float32),
        np.ascontiguousarray(r.reshape(P, 1), np.float32),
    )


def _build(reps=1):
    """Build + compile the single-core Bass program (same program on all cores)."""
    import concourse.bacc as bacc
    import concourse.mybir as mybir
    import concourse.tile as tile

    F32 = mybir.dt.float32
    nc = bacc.Bacc(trn_type="TRN2", target_bir_lowering=False, debug=False)

    x = nc.dram_tensor("x", [BPC, NBLK, P, LANES], F32, kind="ExternalInput").ap()
    w0 = nc.dram_tensor("w0", [P, P], F32, kind="ExternalInput").ap()
    w1 = nc.dram_tensor("w1", [P, P], F32, kind="ExternalInput").ap()
    wf = nc.dram_tensor("wf", [P, P], F32, kind="ExternalInput").ap()
    rv = nc.dram_tensor("rv", [P, 1], F32, kind="ExternalInput").ap()
    y = nc.dram_tensor("y", [BPC, NBLK, P, LANES], F32, kind="ExternalOutput").ap()

    with tile.TileContext(nc) as tc:
        with tc.tile_pool(name="const", bufs=1) as cpool, \
             tc.tile_pool(name="xin", bufs=4) as xpool, \
             tc.tile_pool(name="yout", bufs=3) as ypool, \
             tc.tile_pool(name="ps", bufs=8, space="PSUM") as ppool:

            w0t = cpool.tile([P, P], F32)
            w1t = cpool.tile([P, P], F32)
            wft = cpool.tile([P, P], F32)
            rvt = cpool.tile([P, 1], F32)
            nc.sync.dma_start(out=w0t[:], in_=w0[:])
            nc.sync.dma_start(out=w1t[:], in_=w1[:])
            nc.sync.dma_start(out=wft[:], in_=wf[:])
            nc.sync.dma_start(out=rvt[:], in_=rv[:])

            def body(_=None):
                for pair in range(NPAIR):
                    b0 = pair * PAIR
                    prev = None
                    for s in range(NSUP):
                        k0 = s * SUPER
                        # [P, SUPER, PAIR, LANES] tile; DMA per batch (3-dim APs)
                        xt = xpool.tile([P, SUPER, PAIR, LANES], F32)
                        for bb in range(PAIR):
                            src = x[b0 + bb, k0:k0 + SUPER].rearrange(
                                "k p l -> p k l")
                            nc.sync.dma_start(out=xt[:, :, bb, :], in_=src)
                        ot = ypool.tile([P, SUPER, PAIR, LANES], F32)
                        for i in range(SUPER):
                            blk = k0 + i
                            pt = ppool.tile([P, FREE], F32)
                            rhs = xt[:, i]
                            if blk == 0:
                                nc.tensor.matmul(pt[:], wft[:], rhs,
                                                 start=True, stop=True)
                                nc.vector.tensor_scalar_add(ot[:, i], pt[:], rvt[:])
                            else:
                                prhs = xt[:, i - 1] if i >= 1 else prev[:, SUPER - 1]
                                nc.tensor.matmul(pt[:], w0t[:], rhs,
                                                 start=True, stop=False)
                                nc.tensor.matmul(pt[:], w1t[:], prhs,
                                                 start=False, stop=True)
                                nc.vector.tensor_copy(out=ot[:, i], in_=pt[:])
                        for bb in range(PAIR):
                            dst = y[b0 + bb, k0:k0 + SUPER].rearrange(
                                "k p l -> p k l")
                            nc.sync.dma_start(out=dst, in_=ot[:, :, bb, :])
                        prev = xt

            if reps == 1:
                body()
            else:
                with tc.For_i(0, reps, 1) as _i:
                    body()

    nc.compile()
    return nc


def _in_maps(x, phi, theta_1, theta_2, e_0):
    w0, w1, wf, rv = _coeffs(float(phi[0]), float(theta_1[0]),
                             float(theta_2[0]), float(e_0[0]))
    xs = np.ascontiguousarray(x, np.float32).reshape(NCORES, BPC, NBLK, P, LANES)
    return [
        {"x": xs[i], "w0": w0, "w1": w1, "wf": wf, "rv": rv}
        for i in range(NCORES)
    ]


def kernel(x, phi, theta_1, theta_2, e_0):
    from concourse.bass_utils import run_bass_kernel_spmd

    if "nc" not in _cache:
        _cache["nc"] = _build(reps=1)
    nc = _cache["nc"]
    in_maps = _in_maps(x, phi, theta_1, theta_2, e_0)
    res = run_bass_kernel_spmd(nc, in_maps, core_ids=list(range(NCORES)))
    y = np.stack([res.results[i]["y"] for i in range(NCORES)])
    return y.reshape(B, T, H, W).astype(np.float32)
